# revision 1
# baseline (speedup 1.0000x reference)
"""Trainium2 Bass kernel for GRU + ragged unpad + L2 normalize.

Problem: B=16, T=2048, D=H=1024 single-layer GRU (torch gate order r,z,n),
then per-sequence unpad to flat [sum(lengths), H] and L2-normalize rows.

Sharding: data-parallel over batch, 2 sequences per core across 8 cores.
Per core:
  Phase A: xg = x @ w_ih.T + b_ih   (big GEMM, bf16 operands, fp32 psum)
  Phase B: serial GRU scan over time, per-step hg = h @ w_hh.T via 192
           [128x128]x[128,2] matmuls in transposed layout (gates land on
           128 partitions so DVE/ACT ops are cheap)
  Phase C: L2 normalize each timestep's h vector (partition-dim reduction
           via ones-matmul, sqrt + reciprocal, K=1 ones-matmul broadcast)
Host: pre-transpose x / weights (free), post-transpose + ragged concat.
"""

import numpy as np
import ml_dtypes

B, T, D = 16, 2048, 1024
G3 = 3 * D           # 3072 gate columns
NCORES = 8
BPC = B // NCORES    # 2 sequences per core
KC = D // 128        # 8 contraction chunks
MC = G3 // 128       # 24 output (gate) chunks
HC = D // 128        # 8 hidden chunks
TA = 256             # phase A/C token block
EPS = 1e-12

_cache = {}


def _build(tc_steps: int, tb: int, whh_fp8: bool = True):
    """Build the per-core Bass kernel. tc_steps must be a multiple of tb."""
    import concourse.mybir as mybir
    import concourse.tile as tile
    from concourse import bacc
    from concourse.bass import ds

    f32 = mybir.dt.float32
    bf16 = mybir.dt.bfloat16
    AF = mybir.ActivationFunctionType

    nb = tc_steps // tb
    assert nb * tb == tc_steps
    assert tb % 2 == 0  # h ping-pong parity must match across blocks

    nc = bacc.Bacc("TRN2", enable_partition_id=False)

    xT = nc.dram_tensor("xT", [KC, 128, BPC * T], bf16, kind="ExternalInput")
    wihT = nc.dram_tensor("wihT", [KC, 128, G3], bf16, kind="ExternalInput")
    whh_dt = mybir.dt.float8e4 if whh_fp8 else bf16
    whhT = nc.dram_tensor("whhT", [KC, 128, G3], whh_dt, kind="ExternalInput")
    bih = nc.dram_tensor("bih", [128, MC], f32, kind="ExternalInput")
    bhh = nc.dram_tensor("bhh", [128, MC], f32, kind="ExternalInput")
    yout = nc.dram_tensor("yout", [HC, 128, BPC * T], f32, kind="ExternalOutput")
    # partition-first layouts so the scan's dynamic-offset DMAs can move a
    # whole [128, chunks, BPC, tb] block in a few instructions (each dynamic
    # DMA costs an SP register pair; the register file caps at ~16-31 of them)
    xg_d = nc.dram_tensor("xg_d", [128, MC, BPC, T], f32, kind="Internal")
    y_d = nc.dram_tensor("y_d", [128, HC, BPC, T], f32, kind="Internal")

    n_groups = -(-tc_steps // TA)  # ceil: token blocks per sequence

    with tile.TileContext(nc) as tc:
        with tc.tile_pool(name="persist", bufs=1) as pp:
            wih_sb = pp.tile([128, KC, G3], bf16, tag="wih")
            whh_sb = pp.tile([128, KC, G3], whh_dt, tag="whh")
            bih_sb = pp.tile([128, MC], f32, tag="bih")
            bhh_sb = pp.tile([128, MC], f32, tag="bhh")
            # ping-pong state buffers: all matmuls of step s read slot s%2,
            # gates write slot 1-s%2 (in-place update would leak step-s h into
            # later chunks' matmuls of the same step)
            h_f32 = pp.tile([128, 2, HC, BPC], f32, tag="hf")
            h_bf = pp.tile([128, 2, HC, BPC], bf16, tag="hb")
            ones_k = pp.tile([128, 1], f32, tag="ones_k")
            ones_m = pp.tile([1, 128], f32, tag="ones_m")

            for k in range(KC):
                nc.sync.dma_start(out=wih_sb[:, k, :], in_=wihT[k, :, :])
                nc.sync.dma_start(out=whh_sb[:, k, :], in_=whhT[k, :, :])
            nc.sync.dma_start(out=bih_sb, in_=bih[:, :])
            nc.sync.dma_start(out=bhh_sb, in_=bhh[:, :])
            nc.vector.memset(h_f32, 0.0)
            nc.vector.memset(h_bf, 0.0)
            nc.vector.memset(ones_k, 1.0)
            nc.vector.memset(ones_m, 1.0)

            # ---------------- Phase A: xg = x @ w_ih.T + b_ih ----------------
            with (
                tc.tile_pool(name="pa_x", bufs=3) as pax,
                tc.tile_pool(name="pa_o", bufs=4) as pao,
                tc.tile_pool(name="pa_ps", bufs=2, space="PSUM") as paps,
            ):
                for b in range(BPC):
                    for g in range(n_groups):
                        t0 = g * TA
                        tn = min(TA, tc_steps - t0)
                        xa = pax.tile([128, KC, TA], bf16, tag="xa")
                        for k in range(KC):
                            nc.sync.dma_start(
                                out=xa[:, k, :tn],
                                in_=xT[k, :, b * T + t0 : b * T + t0 + tn],
                            )
                        for m in range(MC):
                            ps = paps.tile([128, TA], f32, tag="ps")
                            for k in range(KC):
                                nc.tensor.matmul(
                                    ps[:, :tn],
                                    wih_sb[:, k, m * 128 : (m + 1) * 128],
                                    xa[:, k, :tn],
                                    start=(k == 0),
                                    stop=(k == KC - 1),
                                )
                            xo = pao.tile([128, TA], f32, tag="xo")
                            nc.scalar.activation(
                                xo[:, :tn], ps[:, :tn], AF.Identity,
                                bias=bih_sb[:, m : m + 1],
                            )
                            nc.sync.dma_start(
                                out=xg_d[:, m, b, t0 : t0 + tn],
                                in_=xo[:, :tn],
                            )

            # ---------------- Phase B: GRU scan ----------------
            with (
                tc.tile_pool(name="pb_xg", bufs=2) as pbx,
                tc.tile_pool(name="pb_y", bufs=2) as pby,
                tc.tile_pool(name="pb_g", bufs=3) as pbg,
                tc.tile_pool(name="pb_r", bufs=2, space="PSUM") as psr,
                tc.tile_pool(name="pb_z", bufs=2, space="PSUM") as psz,
                tc.tile_pool(name="pb_n", bufs=2, space="PSUM") as psn,
            ):
                with tc.For_i(
                    0, nb, 1,
                    hint_engines=(
                        mybir.EngineType.PE,
                        mybir.EngineType.DVE,
                        mybir.EngineType.Activation,
                    ),
                ) as iv:
                    xgb = pbx.tile([128, MC, BPC, tb], f32, tag="xgb")
                    for mg in range(4):
                        m0, m1 = mg * (MC // 4), (mg + 1) * (MC // 4)
                        nc.sync.dma_start(
                            out=xgb[:, m0:m1, :, :],
                            in_=xg_d[:, m0:m1, :, ds(iv * tb, tb)],
                        )
                    yb = pby.tile([128, HC, BPC, tb], f32, tag="yb")
                    for s in range(tb):
                        rd, wr = s % 2, 1 - s % 2
                        for j in range(HC):
                            pr = psr.tile([128, BPC], f32, tag="pr")
                            pz = psz.tile([128, BPC], f32, tag="pz")
                            pn = psn.tile([128, BPC], f32, tag="pn")
                            for k in range(KC):
                                nc.tensor.matmul(
                                    pr, whh_sb[:, k, j * 128 : (j + 1) * 128],
                                    h_bf[:, rd, k, :],
                                    start=(k == 0), stop=(k == KC - 1),
                                )
                            for k in range(KC):
                                nc.tensor.matmul(
                                    pz,
                                    whh_sb[:, k, D + j * 128 : D + (j + 1) * 128],
                                    h_bf[:, rd, k, :],
                                    start=(k == 0), stop=(k == KC - 1),
                                )
                            for k in range(KC):
                                nc.tensor.matmul(
                                    pn,
                                    whh_sb[:, k, 2 * D + j * 128 : 2 * D + (j + 1) * 128],
                                    h_bf[:, rd, k, :],
                                    start=(k == 0), stop=(k == KC - 1),
                                )
                            tr = pbg.tile([128, BPC], f32, tag="tr")
                            nc.vector.tensor_add(tr, pr, xgb[:, j, :, s])
                            r = pbg.tile([128, BPC], f32, tag="r")
                            nc.scalar.activation(
                                r, tr, AF.Sigmoid, bias=bhh_sb[:, j : j + 1]
                            )
                            tz = pbg.tile([128, BPC], f32, tag="tz")
                            nc.vector.tensor_add(tz, pz, xgb[:, HC + j, :, s])
                            z = pbg.tile([128, BPC], f32, tag="z")
                            nc.scalar.activation(
                                z, tz, AF.Sigmoid, bias=bhh_sb[:, HC + j : HC + j + 1]
                            )
                            hn = pbg.tile([128, BPC], f32, tag="hn")
                            nc.scalar.activation(
                                hn, pn, AF.Identity,
                                bias=bhh_sb[:, 2 * HC + j : 2 * HC + j + 1],
                            )
                            tn_ = pbg.tile([128, BPC], f32, tag="tn")
                            nc.vector.tensor_mul(tn_, r, hn)
                            nc.vector.tensor_add(tn_, tn_, xgb[:, 2 * HC + j, :, s])
                            n_ = pbg.tile([128, BPC], f32, tag="n")
                            nc.scalar.activation(n_, tn_, AF.Tanh)
                            d_ = pbg.tile([128, BPC], f32, tag="d")
                            nc.vector.tensor_sub(d_, h_f32[:, rd, j, :], n_)
                            nc.vector.tensor_mul(d_, z, d_)
                            nc.vector.tensor_add(h_f32[:, wr, j, :], n_, d_)
                            nc.vector.tensor_copy(yb[:, j, :, s], h_f32[:, wr, j, :])
                            nc.vector.tensor_copy(h_bf[:, wr, j, :], h_f32[:, wr, j, :])
                    for cg in range(2):
                        c0, c1 = cg * (HC // 2), (cg + 1) * (HC // 2)
                        nc.sync.dma_start(
                            out=y_d[:, c0:c1, :, ds(iv * tb, tb)],
                            in_=yb[:, c0:c1, :, :],
                        )

            # ---------------- Phase C: L2 normalize ----------------
            with (
                tc.tile_pool(name="pc_y", bufs=2) as pcy,
                tc.tile_pool(name="pc_t", bufs=3) as pct,
                tc.tile_pool(name="pc_o", bufs=3) as pco,
                tc.tile_pool(name="pc_ps", bufs=2, space="PSUM") as pcps,
                tc.tile_pool(name="pc_pb", bufs=2, space="PSUM") as pcpb,
            ):
                for b in range(BPC):
                    for g in range(n_groups):
                        t0 = g * TA
                        tn = min(TA, tc_steps - t0)
                        yn = pcy.tile([128, HC, TA], f32, tag="yn")
                        for ch in range(HC):
                            nc.sync.dma_start(
                                out=yn[:, ch, :tn],
                                in_=y_d[:, ch, b, t0 : t0 + tn],
                            )
                        pss = pcps.tile([1, TA], f32, tag="pss")
                        for ch in range(HC):
                            sq = pct.tile([128, TA], f32, tag="sq")
                            nc.vector.tensor_mul(
                                sq[:, :tn], yn[:, ch, :tn], yn[:, ch, :tn]
                            )
                            nc.tensor.matmul(
                                pss[:, :tn], ones_k, sq[:, :tn],
                                start=(ch == 0), stop=(ch == HC - 1),
                            )
                        nrm = pct.tile([1, TA], f32, tag="nrm")
                        nc.scalar.activation(nrm[:, :tn], pss[:, :tn], AF.Sqrt)
                        nc.vector.tensor_scalar_max(nrm[:, :tn], nrm[:, :tn], EPS)
                        rs = pct.tile([1, TA], f32, tag="rs")
                        nc.vector.reciprocal(rs[:, :tn], nrm[:, :tn])
                        psb = pcpb.tile([128, TA], f32, tag="psb")
                        nc.tensor.matmul(
                            psb[:, :tn], ones_m, rs[:, :tn], start=True, stop=True
                        )
                        for ch in range(HC):
                            ysc = pco.tile([128, TA], f32, tag="ysc")
                            nc.vector.tensor_mul(
                                ysc[:, :tn], yn[:, ch, :tn], psb[:, :tn]
                            )
                            nc.sync.dma_start(
                                out=yout[ch, :, b * T + t0 : b * T + t0 + tn],
                                in_=ysc[:, :tn],
                            )

    nc.compile()
    return nc


def _build_noop(whh_fp8: bool = True):
    """Same I/O signature as _build but a trivial body — used by test.py to
    subtract dispatch/transfer overhead from wall-clock timing."""
    import concourse.mybir as mybir
    import concourse.tile as tile
    from concourse import bacc

    f32 = mybir.dt.float32
    bf16 = mybir.dt.bfloat16
    whh_dt = mybir.dt.float8e4 if whh_fp8 else bf16
    nc = bacc.Bacc("TRN2", enable_partition_id=False)
    nc.dram_tensor("xT", [KC, 128, BPC * T], bf16, kind="ExternalInput")
    nc.dram_tensor("wihT", [KC, 128, G3], bf16, kind="ExternalInput")
    nc.dram_tensor("whhT", [KC, 128, G3], whh_dt, kind="ExternalInput")
    bih = nc.dram_tensor("bih", [128, MC], f32, kind="ExternalInput")
    nc.dram_tensor("bhh", [128, MC], f32, kind="ExternalInput")
    yout = nc.dram_tensor("yout", [HC, 128, BPC * T], f32, kind="ExternalOutput")
    with tile.TileContext(nc) as tc:
        with tc.tile_pool(name="p", bufs=1) as p:
            t = p.tile([128, MC], f32, tag="t")
            nc.sync.dma_start(out=t, in_=bih[:, :])
            nc.sync.dma_start(out=yout[0, :, :MC], in_=t)
    nc.compile()
    return nc


def _prep_inputs(x, w_ih, w_hh, b_ih, b_hh, whh_fp8=True):
    """Host-side layout prep (not timed): transposes + dtype casts."""
    bf = ml_dtypes.bfloat16
    whh_dt = ml_dtypes.float8_e4m3 if whh_fp8 else bf
    x = np.asarray(x, dtype=np.float32)
    wihT = np.ascontiguousarray(np.asarray(w_ih, np.float32).T).astype(bf)
    whhT = np.ascontiguousarray(np.asarray(w_hh, np.float32).T).astype(whh_dt)
    wihT = wihT.reshape(KC, 128, G3)
    whhT = whhT.reshape(KC, 128, G3)
    bih = np.ascontiguousarray(
        np.asarray(b_ih, np.float32).reshape(MC, 128).T
    )
    bhh = np.ascontiguousarray(
        np.asarray(b_hh, np.float32).reshape(MC, 128).T
    )
    in_maps = []
    for c in range(NCORES):
        xc = x[c * BPC : (c + 1) * BPC]            # [2, T, D]
        xTc = np.ascontiguousarray(xc.transpose(2, 0, 1))  # [D, 2, T]
        xTc = xTc.reshape(KC, 128, BPC * T).astype(bf)
        in_maps.append(
            {"xT": xTc, "wihT": wihT, "whhT": whhT, "bih": bih, "bhh": bhh}
        )
    return in_maps


def _assemble(results, lengths):
    """Per-core yout [HC,128,BPC*T] fp32 -> flat [sum(lengths), D]."""
    lengths = np.asarray(lengths).astype(np.int64)
    parts = []
    for c in range(NCORES):
        yo = np.asarray(results[c]["yout"], np.float32)
        yo = yo.reshape(D, BPC, T).transpose(1, 2, 0)  # [2, T, D]
        for b in range(BPC):
            parts.append(yo[b, : lengths[c * BPC + b]])
    return np.concatenate(parts, axis=0)


def kernel(x, lengths, w_ih, w_hh, b_ih, b_hh):
    from concourse import bass_utils

    lengths_np = np.asarray(lengths).astype(np.int64)
    max_len = int(lengths_np.max())
    tb = 16
    tc_steps = -(-max_len // tb) * tb
    key = (tc_steps, tb)
    if key not in _cache:
        _cache[key] = _build(tc_steps, tb)
    nc = _cache[key]

    in_maps = _prep_inputs(x, w_ih, w_hh, b_ih, b_hh)
    res = bass_utils.run_bass_kernel_spmd(nc, in_maps, list(range(NCORES)))
    return _assemble(res.results, lengths_np)


if __name__ == "__main__":
    import reference

    inputs = reference.setup_inputs()
    out = kernel(**{k: np.asarray(v) for k, v in inputs.items()})
    exp = np.asarray(reference.reference(**inputs))
    err = np.abs(out - exp).max()
    rel = np.linalg.norm(out - exp) / np.linalg.norm(exp)
    print("absmax:", err, "rel:", rel)



# revision 2
# speedup vs baseline: 1.3975x; 1.3975x over previous
"""Trainium2 Bass kernel for GRU + ragged unpad + L2 normalize — block-Picard.

Key idea: instead of a serial scan with a full [1024x3072] weight sweep per
timestep (192 tiny matmuls/step), process blocks of S timesteps with Picard
fixed-point iteration: freeze the recurrent matvec input at the previous
iterate's trajectory, compute all S steps' gate pre-activations in one wide
GEMM (moving dim = 2 seqs x S steps), then solve the remaining DIAGONAL
linear recurrence h_t = z_t*h_{t-1} + (1-z_t)*n_t exactly with the DVE's
tensor_tensor_scan instruction. Contraction factor ~0.23/iter, so M=6
iterations reach ~1e-4 — far below the fp8/bf16 noise floor.

Per core: 2 sequences (data-parallel over batch across 8 cores).
Everything fused in one pass per block: x DMA -> xg GEMM (biases folded,
bf16) -> M Picard iterations (fp8 weights, identity-matmul folds xg into
the psum for r/z gates) -> L2 normalize -> yout DMA.
"""

import numpy as np
import ml_dtypes

B, T, D = 16, 2048, 1024
G3 = 3 * D
NCORES = 8
BPC = B // NCORES    # 2 sequences per core
KC = D // 128        # 8 contraction chunks
MC = G3 // 128       # 24 gate chunks
HC = D // 128        # 8 hidden chunks
S = 232              # Picard block size (2S <= 512 fits one PSUM bank)
M_ITERS = 5
EPS = 1e-12

_cache = {}


def _build(nb: int, m_iters: int = M_ITERS, s: int = S, repeat: int = 1):
    """repeat>1 wraps the whole body in a For_i hardware loop that re-runs
    the identical computation; used only for differential timing."""
    import concourse.mybir as mybir
    import concourse.tile as tile
    from concourse import bacc
    from contextlib import nullcontext

    f32 = mybir.dt.float32
    bf16 = mybir.dt.bfloat16
    fp8 = mybir.dt.float8e4
    AF = mybir.ActivationFunctionType
    ALU = mybir.AluOpType

    tc = nb * s
    nc = bacc.Bacc("TRN2", enable_partition_id=False)

    xT = nc.dram_tensor("xT", [KC, 128, BPC, tc], bf16, kind="ExternalInput")
    wihT = nc.dram_tensor("wihT", [KC, 128, G3], bf16, kind="ExternalInput")
    whhT = nc.dram_tensor("whhT", [KC, 128, G3], fp8, kind="ExternalInput")
    eye_d = nc.dram_tensor("eye", [128, 128], bf16, kind="ExternalInput")
    biasA_d = nc.dram_tensor("biasA", [128, MC], f32, kind="ExternalInput")
    bhn_d = nc.dram_tensor("bhn", [128, HC], f32, kind="ExternalInput")
    yout = nc.dram_tensor("yout", [HC, 128, BPC, tc], f32, kind="ExternalOutput")

    with tile.TileContext(nc) as tc_:
        with (
            tc_.tile_pool(name="persist", bufs=1) as pp,
            tc_.tile_pool(name="xb", bufs=2) as pxb,
            tc_.tile_pool(name="ps", bufs=4, space="PSUM") as pps,
            tc_.tile_pool(name="psn", bufs=2, space="PSUM") as ppsn,
            tc_.tile_pool(name="pcs", bufs=1, space="PSUM") as pcs,
            tc_.tile_pool(name="pcb", bufs=1, space="PSUM") as pcb,
            tc_.tile_pool(name="sq", bufs=2) as psq,
            tc_.tile_pool(name="yo", bufs=2) as pyo,
        ):
            wih_sb = pp.tile([128, KC, G3], bf16, tag="wih")
            whh_sb = pp.tile([128, KC, G3], fp8, tag="whh")
            eye = pp.tile([128, 128], bf16, tag="eye")
            biasA = pp.tile([128, MC], f32, tag="biasA")
            bhn = pp.tile([128, HC], f32, tag="bhn")
            # Picard trajectory ping-pong: slot 0 = block-entry h, 1..S = steps
            HA = pp.tile([128, KC, BPC, s + 1], bf16, tag="HA")
            HB = pp.tile([128, KC, BPC, s + 1], bf16, tag="HB")
            hent = pp.tile([128, KC, BPC, 1], f32, tag="hent")
            xgb = pp.tile([128, MC, BPC, s], bf16, tag="xgb")
            zt = pp.tile([128, HC, BPC, s], bf16, tag="zt")
            rt = pp.tile([128, HC, BPC, s], bf16, tag="rt")
            nt = pp.tile([128, HC, BPC, s], bf16, tag="nt")
            nt2 = pp.tile([128, HC, BPC, s], bf16, tag="nt2")
            zs = pp.tile([128, s], bf16, tag="zs")
            ones_k = pp.tile([128, 1], bf16, tag="ones_k")
            ones_m = pp.tile([1, 128], bf16, tag="ones_m")
            rsb = pp.tile([1, s], bf16, tag="rsb")
            nrm = pp.tile([1, s], f32, tag="nrm")
            rsf = pp.tile([1, s], f32, tag="rsf")

            for k in range(KC):
                nc.sync.dma_start(out=wih_sb[:, k, :], in_=wihT[k, :, :])
                nc.sync.dma_start(out=whh_sb[:, k, :], in_=whhT[k, :, :])
            nc.sync.dma_start(out=eye, in_=eye_d[:, :])
            nc.sync.dma_start(out=biasA, in_=biasA_d[:, :])
            nc.sync.dma_start(out=bhn, in_=bhn_d[:, :])
            nc.vector.memset(zs, 0.0)
            nc.vector.memset(ones_k, 1.0)
            nc.vector.memset(ones_m, 1.0)
            nc.vector.memset(HA[:, :, :, 0:1], 0.0)
            nc.vector.memset(HB[:, :, :, 0:1], 0.0)
            nc.vector.memset(hent, 0.0)

            bufs = [HA, HB]
            rep_ctx = (
                tc_.For_i(
                    0, repeat, 1,
                    hint_engines=(
                        mybir.EngineType.PE,
                        mybir.EngineType.DVE,
                        mybir.EngineType.Activation,
                    ),
                )
                if repeat > 1 else nullcontext()
            )
            with rep_ctx:
                _body(nc, tile, mybir, nb, m_iters, s, bufs, locals())

    nc.compile()
    return nc


def _never():  # placeholder to keep indentation sane
    pass


def _body(nc, tile, mybir, nb, m_iters, s, bufs, env):
    f32 = mybir.dt.float32
    bf16 = mybir.dt.bfloat16
    AF = mybir.ActivationFunctionType
    ALU = mybir.AluOpType
    (HA, HB, hent, xgb, zt, rt, nt, nt2, zs, ones_k, ones_m, rsb, nrm, rsf,
     wih_sb, whh_sb, eye, biasA, bhn, xT, yout, pxb, pps, ppsn, pcs, pcb,
     psq, pyo) = (
        env["HA"], env["HB"], env["hent"], env["xgb"], env["zt"], env["rt"],
        env["nt"], env["nt2"], env["zs"], env["ones_k"], env["ones_m"],
        env["rsb"], env["nrm"], env["rsf"], env["wih_sb"], env["whh_sb"],
        env["eye"], env["biasA"], env["bhn"], env["xT"], env["yout"],
        env["pxb"], env["pps"], env["ppsn"], env["pcs"], env["pcb"],
        env["psq"], env["pyo"],
    )
    for blk in range(nb):
                t0 = blk * s
                if blk > 0:
                    # entry h := final h of previous block (in HY)
                    HY = bufs[m_iters % 2]
                    nc.vector.tensor_copy(HA[:, :, :, 0:1], HY[:, :, :, s:s+1])
                    nc.vector.tensor_copy(HB[:, :, :, 0:1], HY[:, :, :, s:s+1])
                    nc.vector.tensor_copy(hent, HY[:, :, :, s:s+1])
                # iteration-0 trajectory: constant = entry h
                for j in range(KC):
                    for b in range(BPC):
                        nc.vector.tensor_scalar_add(
                            HA[:, j, b, 1:s+1], zs, hent[:, j, b, :]
                        )

                xb = pxb.tile([128, KC, BPC, s], bf16, tag="xb")
                for k in range(KC):
                    nc.sync.dma_start(
                        out=xb[:, k, :, :], in_=xT[k, :, :, t0:t0+s]
                    )
                # xg = x @ w_ih.T + biasA (bih, + bhh for r/z), bf16
                for m in range(MC):
                    ps = pps.tile([128, BPC, s], f32, tag="ps")
                    for k in range(KC):
                        nc.tensor.matmul(
                            ps, wih_sb[:, k, m*128:(m+1)*128], xb[:, k, :, :],
                            start=(k == 0), stop=(k == KC - 1),
                        )
                    nc.scalar.activation(
                        xgb[:, m, :, :], ps, AF.Identity,
                        bias=biasA[:, m:m+1],
                    )

                for it in range(m_iters):
                    HR = bufs[it % 2]
                    HW = bufs[1 - it % 2]
                    for j in range(HC):
                        # r gate (m-chunk j), xg folded via identity matmul
                        pr = pps.tile([128, BPC, s], f32, tag="ps")
                        for k in range(KC):
                            nc.tensor.matmul(
                                pr, whh_sb[:, k, j*128:(j+1)*128],
                                HR[:, k, :, 0:s],
                                start=(k == 0), stop=False,
                            )
                        nc.tensor.matmul(
                            pr, eye, xgb[:, j, :, :], start=False, stop=True,
                        )
                        nc.scalar.activation(rt[:, j, :, :], pr, AF.Sigmoid)
                        # z gate (m-chunk 8+j)
                        pz = pps.tile([128, BPC, s], f32, tag="ps")
                        for k in range(KC):
                            nc.tensor.matmul(
                                pz, whh_sb[:, k, D + j*128:D + (j+1)*128],
                                HR[:, k, :, 0:s],
                                start=(k == 0), stop=False,
                            )
                        nc.tensor.matmul(
                            pz, eye, xgb[:, HC + j, :, :],
                            start=False, stop=True,
                        )
                        nc.scalar.activation(zt[:, j, :, :], pz, AF.Sigmoid)
                        # n pre-activation (m-chunk 16+j), no xg fold
                        pn = ppsn.tile([128, BPC, s], f32, tag="psn")
                        for k in range(KC):
                            nc.tensor.matmul(
                                pn, whh_sb[:, k, 2*D + j*128:2*D + (j+1)*128],
                                HR[:, k, :, 0:s],
                                start=(k == 0), stop=(k == KC - 1),
                            )
                        # t = (g_n + bhn_j) * r  (PSUM-in0 stt wedges the HW,
                        # so drain via ACT identity+bias first)
                        nc.scalar.activation(
                            nt[:, j, :, :], pn, AF.Identity,
                            bias=bhn[:, j:j+1],
                        )
                        nc.vector.tensor_mul(
                            nt[:, j, :, :], nt[:, j, :, :], rt[:, j, :, :]
                        )
                        nc.vector.tensor_add(
                            nt[:, j, :, :], nt[:, j, :, :],
                            xgb[:, 2*HC + j, :, :],
                        )
                        nc.scalar.activation(
                            nt2[:, j, :, :], nt[:, j, :, :], AF.Tanh
                        )
                        # nb = (z - 1) * n   (so h = z*h - nb)
                        nc.vector.scalar_tensor_tensor(
                            nt[:, j, :, :], zt[:, j, :, :], -1.0,
                            nt2[:, j, :, :], ALU.add, ALU.mult,
                        )
                        for b in range(BPC):
                            nc.vector.tensor_tensor_scan(
                                HW[:, j, b, 1:s+1], zt[:, j, b, :],
                                nt[:, j, b, :], hent[:, j, b, :],
                                ALU.mult, ALU.subtract,
                            )

                # normalize + emit this block from the final buffer HY
                HY = bufs[m_iters % 2]
                for b in range(BPC):
                    pss = pcs.tile([1, s], f32, tag="pss")
                    hks = []
                    for k in range(KC):
                        hk = psq.tile([128, s], f32, tag=f"hk{k}")
                        nc.vector.tensor_copy(hk, HY[:, k, b, 1:s+1])
                        hks.append(hk)
                        sq = psq.tile([128, s], bf16, tag="sq")
                        nc.vector.tensor_mul(sq, hk, hk)
                        nc.tensor.matmul(
                            pss, ones_k, sq,
                            start=(k == 0), stop=(k == KC - 1),
                        )
                    nc.scalar.activation(nrm, pss, AF.Sqrt)
                    nc.vector.tensor_scalar_max(nrm, nrm, EPS)
                    nc.vector.reciprocal(rsf, nrm)
                    nc.vector.tensor_copy(rsb, rsf)
                    pbc = pcb.tile([128, s], f32, tag="pbc")
                    nc.tensor.matmul(pbc, ones_m, rsb, start=True, stop=True)
                    for k in range(KC):
                        yo = pyo.tile([128, s], f32, tag="yo")
                        nc.vector.tensor_mul(yo, hks[k], pbc)
                        nc.sync.dma_start(
                            out=yout[k, :, b, t0:t0+s], in_=yo
                        )


def _build_noop(nb: int):
    """Same I/O signature as _build but a trivial body — used by test2.py to
    subtract dispatch/transfer overhead from wall-clock timing."""
    import concourse.mybir as mybir
    import concourse.tile as tile
    from concourse import bacc

    f32 = mybir.dt.float32
    bf16 = mybir.dt.bfloat16
    fp8 = mybir.dt.float8e4
    tc = nb * S
    nc = bacc.Bacc("TRN2", enable_partition_id=False)
    nc.dram_tensor("xT", [KC, 128, BPC, tc], bf16, kind="ExternalInput")
    nc.dram_tensor("wihT", [KC, 128, G3], bf16, kind="ExternalInput")
    nc.dram_tensor("whhT", [KC, 128, G3], fp8, kind="ExternalInput")
    nc.dram_tensor("eye", [128, 128], bf16, kind="ExternalInput")
    biasA = nc.dram_tensor("biasA", [128, MC], f32, kind="ExternalInput")
    nc.dram_tensor("bhn", [128, HC], f32, kind="ExternalInput")
    yout = nc.dram_tensor("yout", [HC, 128, BPC, tc], f32,
                          kind="ExternalOutput")
    with tile.TileContext(nc) as tc_:
        with tc_.tile_pool(name="p", bufs=1) as p:
            t = p.tile([128, MC], f32, tag="t")
            nc.sync.dma_start(out=t, in_=biasA[:, :])
            nc.sync.dma_start(out=yout[0, :, 0, :MC], in_=t)
    nc.compile()
    return nc


def _prep_inputs(x, w_ih, w_hh, b_ih, b_hh, tc):
    """Host-side layout prep (not timed): transposes + dtype casts."""
    bf = ml_dtypes.bfloat16
    f8 = ml_dtypes.float8_e4m3
    x = np.asarray(x, np.float32)
    wihT = np.ascontiguousarray(np.asarray(w_ih, np.float32).T).astype(bf)
    whhT = np.ascontiguousarray(np.asarray(w_hh, np.float32).T).astype(f8)
    wihT = wihT.reshape(KC, 128, G3)
    whhT = whhT.reshape(KC, 128, G3)
    biasA = np.asarray(b_ih, np.float32).copy()
    biasA[:2*D] += np.asarray(b_hh, np.float32)[:2*D]
    biasA = np.ascontiguousarray(biasA.reshape(MC, 128).T)
    bhn = np.ascontiguousarray(
        np.asarray(b_hh, np.float32)[2*D:].reshape(HC, 128).T
    )
    eye = np.eye(128, dtype=np.float32).astype(bf)
    in_maps = []
    for c in range(NCORES):
        xc = x[c*BPC:(c+1)*BPC, :tc]                  # [2, tc, D]
        xTc = np.ascontiguousarray(xc.transpose(2, 0, 1))  # [D, 2, tc]
        xTc = xTc.reshape(KC, 128, BPC, tc).astype(bf)
        in_maps.append({
            "xT": xTc, "wihT": wihT, "whhT": whhT, "eye": eye,
            "biasA": biasA, "bhn": bhn,
        })
    return in_maps


def _assemble(results, lengths, tc):
    """Per-core yout [HC,128,BPC,tc] f32 -> flat [sum(lengths), D]."""
    lengths = np.asarray(lengths).astype(np.int64)
    parts = []
    for c in range(NCORES):
        yo = np.asarray(results[c]["yout"], np.float32)
        yo = yo.reshape(D, BPC, tc).transpose(1, 2, 0)  # [2, tc, D]
        for b in range(BPC):
            parts.append(yo[b, :lengths[c*BPC + b]])
    return np.concatenate(parts, axis=0)


def kernel(x, lengths, w_ih, w_hh, b_ih, b_hh):
    from concourse import bass_utils

    lengths_np = np.asarray(lengths).astype(np.int64)
    max_len = int(lengths_np.max())
    nb = -(-max_len // S)
    tc = nb * S
    key = (nb, M_ITERS, S)
    if key not in _cache:
        _cache[key] = _build(nb)
    nc = _cache[key]

    in_maps = _prep_inputs(x, w_ih, w_hh, b_ih, b_hh, tc)
    res = bass_utils.run_bass_kernel_spmd(nc, in_maps, list(range(NCORES)))
    return _assemble(res.results, lengths_np, tc)


if __name__ == "__main__":
    import reference

    inputs = reference.setup_inputs()
    out = kernel(**{k: np.asarray(v) for k, v in inputs.items()})
    exp = np.asarray(reference.reference(**inputs))
    err = np.abs(out - exp).max()
    rel = np.linalg.norm(out - exp) / np.linalg.norm(exp)
    print("absmax:", err, "rel:", rel)
